# revision 8
# baseline (speedup 1.0000x reference)
"""Trainium2 Bass kernel for nn_ClassLoss (YOLO-style classification CE loss).

Strategy: the loss depends only on grid cells hit by valid target boxes
(<=50 cells/batch out of 4096). Each cell corresponds to 3 consecutive
"flat rows" of the [12288, 85] logits block (765 contiguous floats in DRAM).
So instead of streaming 127MB of logits, each core:
  1. loads its 4 batches' targets [50,5],
  2. computes per-box (row, col, class, valid), resolves last-write-wins
     duplicates with a [50,50] pairwise comparison,
  3. indirect-DMA-gathers the 50 cell blocks [50, 255] per batch,
  4. computes logsumexp over the 80 classes for the 3 rows of each cell and
     the label logit via a one-hot dot, masked by the winner flags,
  5. reduces to a per-core partial sum of per-batch mean CE losses.
Host sums the 8 per-core partials and divides by the global batch size.
"""

import sys

sys.path.insert(0, "/opt/trn_rl_repo")

import numpy as np

import concourse.bass as bass
import concourse.tile as tile
from concourse import bacc, mybir
from concourse.bass_utils import run_bass_kernel_spmd

# Problem constants (hardcoded per harness contract).
B, A, H, W, NC_CLS, M = 32, 3, 64, 64, 80, 50
N_CORES = 8
B_CORE = B // N_CORES          # 4 batches per core
CELLS = H * W                  # 4096 cells per batch
ROWLEN = 3 * (5 + NC_CLS)      # 255 floats per cell (3 anchor rows x 85)
FP32 = mybir.dt.float32
I32 = mybir.dt.int32
Alu = mybir.AluOpType
Act = mybir.ActivationFunctionType


def _host_consts():
    # cidx[*, a*85 + k] = k-5 for k in [5,85), else -1 (never matches a class)
    cidx = np.full((128, ROWLEN), -1.0, dtype=np.float32)
    for a in range(3):
        cidx[:, a * 85 + 5 : (a + 1) * 85] = np.arange(NC_CLS, dtype=np.float32)
    # ut[m, m'] = 1 if m' > m (strictly-later box)
    ut = np.triu(np.ones((M, M), dtype=np.float32), k=1)
    ident = np.eye(128, dtype=np.float32)
    return {"cidx": cidx, "ut": ut, "ident": ident}


def _build_kernel_body(tc, x_ap, t_ap, loss_ap, cidx_ap, ut_ap, ident_ap):
    nc = tc.nc
    from contextlib import ExitStack

    ctx = ExitStack()
    with ctx:
        consts = ctx.enter_context(tc.tile_pool(name="consts", bufs=1))
        work = ctx.enter_context(tc.tile_pool(name="work", bufs=4))
        gpool = ctx.enter_context(tc.tile_pool(name="gather", bufs=4))
        psum = ctx.enter_context(tc.tile_pool(name="psum", bufs=2, space="PSUM"))
        psumr = ctx.enter_context(tc.tile_pool(name="psumr", bufs=1, space="PSUM"))
        fpool = ctx.enter_context(tc.tile_pool(name="final", bufs=1))

        # ---- constants / persistent tiles ----
        cidx_t = consts.tile([128, ROWLEN], FP32)
        nc.sync.dma_start(cidx_t[:], cidx_ap[:])
        ut_t = consts.tile([M, M], FP32)
        nc.sync.dma_start(ut_t[:], ut_ap[:])
        ident_t = consts.tile([128, 128], FP32)
        nc.sync.dma_start(ident_t[:], ident_ap[:])
        ones_t = consts.tile([128, 1], FP32)
        nc.gpsimd.memset(ones_t[:], 1.0)

        stats = fpool.tile([128, 2 * B_CORE], FP32)  # (num_b, cnt_b) x 4
        nc.vector.memset(stats[:], 0.0)

        # all targets for this core: [50, 4, 5]
        tgt_t = consts.tile([M, B_CORE * 5], FP32)
        nc.sync.dma_start(
            tgt_t[:].rearrange("p (b f) -> p b f", f=5),
            t_ap.rearrange("b m f -> m b f"),
        )

        for b in range(B_CORE):
            Tb = tgt_t[:, b * 5 : (b + 1) * 5]
            cls = Tb[:, 0:1]

            # valid[m] = sum(|t|) > 0
            val1 = work.tile([M, 1], FP32, tag="val1")
            nc.vector.tensor_reduce(
                val1[:], Tb, axis=mybir.AxisListType.X, op=Alu.add,
                apply_absolute_value=True,
            )
            valid = work.tile([M, 1], FP32, tag="valid")
            nc.vector.tensor_scalar(valid[:], val1[:], 0.0, None, op0=Alu.is_gt)

            # c = floor(x*64), r = floor(y*64).
            # Exact branchless floor for v in [0, 2^22):
            #   ri = (v + 2^23) - 2^23  (round-to-nearest int via fp32 precision)
            #   floor = ri - (ri > v)
            MAGIC = 8388608.0  # 2^23

            def _floor64(src, name):
                v = work.tile([M, 1], FP32, tag=f"v_{name}")
                nc.vector.tensor_scalar(v[:], src, 64.0, None, op0=Alu.mult)
                ri = work.tile([M, 1], FP32, tag=f"ri_{name}")
                nc.vector.tensor_scalar(ri[:], v[:], MAGIC, None, op0=Alu.add)
                nc.vector.tensor_scalar(ri[:], ri[:], MAGIC, None, op0=Alu.subtract)
                corr = work.tile([M, 1], FP32, tag=f"corr_{name}")
                nc.vector.tensor_tensor(corr[:], ri[:], v[:], op=Alu.is_gt)
                fl = work.tile([M, 1], FP32, tag=f"fl_{name}")
                nc.vector.tensor_tensor(fl[:], ri[:], corr[:], op=Alu.subtract)
                return fl

            cc = _floor64(Tb[:, 1:2], "c")
            rr = _floor64(Tb[:, 2:3], "r")

            # cell = r*64 + c + b*4096  (invalid boxes are all-zero -> cell b*4096)
            cellf = work.tile([M, 1], FP32, tag="cellf")
            nc.vector.scalar_tensor_tensor(
                cellf[:], rr[:], 64.0, cc[:], op0=Alu.mult, op1=Alu.add
            )
            celli = work.tile([M, 1], I32, tag="celli")
            nc.vector.tensor_scalar(celli[:], cellf[:], float(b * CELLS), None, op0=Alu.add)

            # ---- winner resolution (last valid write wins) ----
            # r_eff = valid ? r : -1 so invalid boxes never match any cell
            reff = work.tile([M, 1], FP32, tag="reff")
            nc.vector.scalar_tensor_tensor(
                reff[:], rr[:], 1.0, valid[:], op0=Alu.add, op1=Alu.mult
            )
            nc.vector.tensor_scalar(reff[:], reff[:], -1.0, None, op0=Alu.add)

            # transposed broadcasts: qT[m, m'] = (r_eff[m'], c[m'])
            qT = psum.tile([M, 2 * M], FP32, tag="qT", space="PSUM")
            nc.tensor.transpose(
                qT[:, 0:M], reff[:].to_broadcast([M, M]), ident_t[:M, :M]
            )
            nc.tensor.transpose(
                qT[:, M : 2 * M], cc[:].to_broadcast([M, M]), ident_t[:M, :M]
            )

            sameR = work.tile([M, M], FP32, tag="sameR")
            nc.vector.tensor_scalar(sameR[:], qT[:, 0:M], reff[:], None, op0=Alu.is_equal)
            sameRC = work.tile([M, M], FP32, tag="sameRC")
            nc.vector.scalar_tensor_tensor(
                sameRC[:], qT[:, M : 2 * M], cc[:], sameR[:],
                op0=Alu.is_equal, op1=Alu.mult,
            )
            scrap0 = work.tile([M, M], FP32, tag="scrap0")
            coll = work.tile([M, 1], FP32, tag="coll")
            nc.vector.tensor_tensor(scrap0[:], sameRC[:], ut_t[:], op=Alu.mult)
            nc.vector.tensor_reduce(
                coll[:], scrap0[:], axis=mybir.AxisListType.X, op=Alu.add
            )
            winner = work.tile([M, 1], FP32, tag="winner")
            nc.vector.scalar_tensor_tensor(
                winner[:], coll[:], 0.0, valid[:], op0=Alu.is_equal, op1=Alu.mult
            )

            # ---- gather the 50 cell blocks: [50, 255] ----
            graw = gpool.tile([M, ROWLEN], FP32, tag="graw")
            nc.gpsimd.indirect_dma_start(
                out=graw[:],
                out_offset=None,
                in_=x_ap,
                in_offset=bass.IndirectOffsetOnAxis(ap=celli[:, :1], axis=0),
            )

            # ---- per-cell CE pieces ----
            gv = graw[:].rearrange("p (a f) -> p a f", a=3)[:, :, 5:]
            ex = gpool.tile([M, 3 * NC_CLS], FP32, tag="ex")
            nc.scalar.activation(
                ex[:].rearrange("p (a f) -> p a f", f=NC_CLS), gv, Act.Exp
            )
            se = work.tile([M, 3], FP32, tag="se")
            nc.vector.tensor_reduce(
                se[:], ex[:].rearrange("p (a f) -> p a f", f=NC_CLS),
                axis=mybir.AxisListType.X, op=Alu.add,
            )
            lse = work.tile([M, 3], FP32, tag="lse")
            nc.scalar.activation(lse[:], se[:], Act.Ln)
            s3 = work.tile([M, 1], FP32, tag="s3")
            nc.vector.tensor_reduce(
                s3[:], lse[:], axis=mybir.AxisListType.X, op=Alu.add
            )

            # label logit sum over the 3 rows: one-hot dot against cidx
            ohc = work.tile([M, ROWLEN], FP32, tag="ohc")
            nc.vector.tensor_scalar(ohc[:], cidx_t[:M, :], cls, None, op0=Alu.is_equal)
            scrap1 = work.tile([M, ROWLEN], FP32, tag="scrap1")
            g3 = work.tile([M, 1], FP32, tag="g3")
            nc.vector.tensor_tensor(scrap1[:], ohc[:], graw[:], op=Alu.mult)
            nc.vector.tensor_reduce(
                g3[:], scrap1[:], axis=mybir.AxisListType.X, op=Alu.add
            )

            # d = (lse_sum - label_logit_sum); num_b += winner*d ; cnt_b += winner
            d = work.tile([M, 1], FP32, tag="d")
            nc.vector.tensor_tensor(d[:], s3[:], g3[:], op=Alu.subtract)
            nc.vector.tensor_tensor(
                stats[:M, 2 * b : 2 * b + 1], d[:], winner[:], op=Alu.mult
            )
            nc.vector.tensor_copy(stats[:M, 2 * b + 1 : 2 * b + 2], winner[:])

        # ---- cross-partition reduce via PE: [1, 8] = ones^T @ stats ----
        red = psumr.tile([1, 2 * B_CORE], FP32, tag="red", space="PSUM")
        nc.tensor.matmul(red[:], ones_t[:], stats[:], start=True, stop=True)

        fin = fpool.tile([1, 2 * B_CORE], FP32)
        nc.vector.tensor_copy(fin[:], red[:])
        finv = fin[:].rearrange("p (b t) -> p b t", t=2)
        nums = finv[:, :, 0:1]
        cnts = finv[:, :, 1:2]
        den = fpool.tile([1, B_CORE], FP32)
        nc.vector.tensor_scalar(den[:, :, None], cnts, 3.0, None, op0=Alu.mult)
        nc.vector.tensor_scalar(den[:], den[:], 1.0, None, op0=Alu.max)
        rden = fpool.tile([1, B_CORE], FP32)
        nc.vector.reciprocal(rden[:], den[:])
        lb = fpool.tile([1, B_CORE], FP32)
        nc.vector.tensor_tensor(lb[:, :, None], nums, rden[:, :, None], op=Alu.mult)
        lsum = fpool.tile([1, 1], FP32)
        nc.vector.tensor_reduce(
            lsum[:], lb[:], axis=mybir.AxisListType.X, op=Alu.add
        )
        nc.sync.dma_start(loss_ap[:], lsum[:])


_CACHE = {}


def _get_compiled():
    if "nc" in _CACHE:
        return _CACHE["nc"]
    nc = bacc.Bacc(
        "TRN2",
        target_bir_lowering=False,
        debug=False,
        enable_asserts=False,
        num_devices=N_CORES,
    )
    x = nc.dram_tensor("xflat", [B_CORE * CELLS, ROWLEN], FP32, kind="ExternalInput")
    t = nc.dram_tensor("tgt", [B_CORE, M, 5], FP32, kind="ExternalInput")
    cidx = nc.dram_tensor("cidx", [128, ROWLEN], FP32, kind="ExternalInput")
    ut = nc.dram_tensor("ut", [M, M], FP32, kind="ExternalInput")
    ident = nc.dram_tensor("ident", [128, 128], FP32, kind="ExternalInput")
    loss = nc.dram_tensor("loss", [1, 1], FP32, kind="ExternalOutput")

    with tile.TileContext(nc) as tc:
        _build_kernel_body(
            tc, x.ap(), t.ap(), loss.ap(), cidx.ap(), ut.ap(), ident.ap()
        )
    nc.compile()
    _CACHE["nc"] = nc
    return nc


def _run(output, targets, trace=False):
    nc = _get_compiled()
    consts = _host_consts()
    output = np.ascontiguousarray(output, dtype=np.float32)
    targets = np.ascontiguousarray(targets, dtype=np.float32)
    in_maps = []
    for k in range(N_CORES):
        in_maps.append(
            {
                "xflat": output[k * B_CORE : (k + 1) * B_CORE].reshape(
                    B_CORE * CELLS, ROWLEN
                ),
                "tgt": targets[k * B_CORE : (k + 1) * B_CORE],
                **consts,
            }
        )
    res = run_bass_kernel_spmd(nc, in_maps, core_ids=list(range(N_CORES)), trace=trace)
    total = sum(float(r["loss"][0, 0]) for r in res.results)
    return np.float32(total / B), res


def kernel(output, targets):
    val, _ = _run(output, targets)
    return np.asarray(val, dtype=np.float32)


# revision 9
# speedup vs baseline: 1.2201x; 1.2201x over previous
"""Trainium2 Bass kernel for nn_ClassLoss (YOLO-style classification CE loss).

Strategy: the loss depends only on grid cells hit by valid target boxes
(<=50 cells/batch out of 4096). Each cell corresponds to 3 consecutive
"flat rows" of the [12288, 85] logits block (765 contiguous floats in DRAM).
So instead of streaming 127MB of logits, each core:
  1. loads its 4 batches' targets,
  2. computes per-box (row, col, class, valid), resolves last-write-wins
     duplicates with a pairwise comparison (block-diagonal across batches),
  3. indirect-DMA-gathers the needed cell blocks (two [100, 255] gathers,
     batches stacked in pairs along the partition axis),
  4. computes logsumexp over the 80 classes for the 3 rows of each cell and
     the label logit via a one-hot dot, masked by the winner flags,
  5. reduces to per-batch (loss_sum, cell_count) pairs via a selector matmul.
Host applies the per-batch mean (num / max(3*cnt,1)), sums across cores and
divides by the global batch size (the all-reduce + normalize of the
data-parallel sharding).
"""

import sys

sys.path.insert(0, "/opt/trn_rl_repo")

import numpy as np

import concourse.bass as bass
import concourse.tile as tile
from concourse import bacc, mybir
from concourse.bass_utils import run_bass_kernel_spmd

# Problem constants (hardcoded per harness contract).
B, A, H, W, NC_CLS, M = 32, 3, 64, 64, 80, 50
N_CORES = 8
B_CORE = B // N_CORES          # 4 batches per core
CELLS = H * W                  # 4096 cells per batch
ROWLEN = 3 * (5 + NC_CLS)      # 255 floats per cell (3 anchor rows x 85)
P2 = 2 * M                     # 100 partitions: 2 batches x 50 boxes
FP32 = mybir.dt.float32
I32 = mybir.dt.int32
Alu = mybir.AluOpType
Act = mybir.ActivationFunctionType


def _host_consts():
    # cidx[*, a*85 + k] = k-5 for k in [5,85), else -1 (never matches a class)
    cidx = np.full((P2, ROWLEN), -1.0, dtype=np.float32)
    for a in range(3):
        cidx[:, a * 85 + 5 : (a + 1) * 85] = np.arange(NC_CLS, dtype=np.float32)
    # ut2[p, q] = 1 iff same 50-block and q%50 > p%50 (strictly-later box)
    blk = np.arange(P2) // M
    mi = np.arange(P2) % M
    ut2 = ((blk[:, None] == blk[None, :]) & (mi[None, :] > mi[:, None])).astype(
        np.float32
    )
    ident = np.eye(P2, dtype=np.float32)
    # cell offset per partition, per pair: batch = 2*j + p//50
    boff = np.empty((P2, 2), dtype=np.float32)
    for j in range(2):
        boff[:M, j] = (2 * j) * CELLS
        boff[M:, j] = (2 * j + 1) * CELLS
    # block selector for per-batch partition sums
    bsel = np.zeros((P2, 2), dtype=np.float32)
    bsel[:M, 0] = 1.0
    bsel[M:, 1] = 1.0
    return {"cidx": cidx, "ut2": ut2, "ident": ident, "boff": boff, "bsel": bsel}


def _build_kernel_body(tc, x_ap, t_ap, out_ap, cidx_ap, ut_ap, ident_ap, boff_ap, bsel_ap):
    nc = tc.nc
    from contextlib import ExitStack

    ctx = ExitStack()
    with ctx:
        consts = ctx.enter_context(tc.tile_pool(name="consts", bufs=1))
        work = ctx.enter_context(tc.tile_pool(name="work", bufs=3))
        gpool = ctx.enter_context(tc.tile_pool(name="gather", bufs=2))
        psum = ctx.enter_context(tc.tile_pool(name="psum", bufs=2, space="PSUM"))
        psumr = ctx.enter_context(tc.tile_pool(name="psumr", bufs=1, space="PSUM"))
        fpool = ctx.enter_context(tc.tile_pool(name="final", bufs=1))

        # ---- constants / persistent tiles ----
        cidx_t = consts.tile([P2, ROWLEN], FP32)
        nc.sync.dma_start(cidx_t[:], cidx_ap[:])
        ut_t = consts.tile([P2, P2], FP32)
        nc.sync.dma_start(ut_t[:], ut_ap[:])
        ident_t = consts.tile([P2, P2], FP32)
        nc.sync.dma_start(ident_t[:], ident_ap[:])
        boff_t = consts.tile([P2, 2], FP32)
        nc.sync.dma_start(boff_t[:], boff_ap[:])
        bsel_t = consts.tile([P2, 2], FP32)
        nc.sync.dma_start(bsel_t[:], bsel_ap[:])

        stats = fpool.tile([P2, 4], FP32)  # (num, cnt) per pair-column

        # all targets: [100, 2, 5]; partition p = batch-in-pair p//50, box p%50
        tgt_t = consts.tile([P2, 2 * 5], FP32)
        nc.sync.dma_start(
            tgt_t[:].rearrange("p (j f) -> p j f", f=5),
            t_ap.rearrange("(j bb) m f -> (bb m) j f", j=2),
        )

        MAGIC = 8388608.0  # 2^23

        for j in range(2):
            Tb = tgt_t[:].rearrange("p (j f) -> p j f", f=5)[:, j, :]
            cls = Tb[:, 0:1]

            # valid[m] = sum(|t|) > 0
            val1 = work.tile([P2, 1], FP32, tag="val1")
            nc.vector.tensor_reduce(
                val1[:], Tb, axis=mybir.AxisListType.X, op=Alu.add,
                apply_absolute_value=True,
            )
            valid = work.tile([P2, 1], FP32, tag="valid")
            nc.vector.tensor_scalar(valid[:], val1[:], 0.0, None, op0=Alu.is_gt)

            # c = floor(x*64), r = floor(y*64): exact branchless floor via
            # ri = RNE(v) (magic add/sub), floor = ri - (ri > v)
            def _floor64(src, name):
                v = work.tile([P2, 1], FP32, tag=f"v_{name}")
                nc.vector.tensor_scalar(v[:], src, 64.0, None, op0=Alu.mult)
                ri = work.tile([P2, 1], FP32, tag=f"ri_{name}")
                nc.vector.tensor_scalar(ri[:], v[:], MAGIC, None, op0=Alu.add)
                nc.vector.tensor_scalar(ri[:], ri[:], MAGIC, None, op0=Alu.subtract)
                corr = work.tile([P2, 1], FP32, tag=f"corr_{name}")
                nc.vector.tensor_tensor(corr[:], ri[:], v[:], op=Alu.is_gt)
                fl = work.tile([P2, 1], FP32, tag=f"fl_{name}")
                nc.vector.tensor_tensor(fl[:], ri[:], corr[:], op=Alu.subtract)
                return fl

            cc = _floor64(Tb[:, 1:2], "c")
            rr = _floor64(Tb[:, 2:3], "r")

            # cell = r*64 + c + batch_offset
            cellf = work.tile([P2, 1], FP32, tag="cellf")
            nc.vector.scalar_tensor_tensor(
                cellf[:], rr[:], 64.0, cc[:], op0=Alu.mult, op1=Alu.add
            )
            celli = work.tile([P2, 1], I32, tag="celli")
            nc.vector.tensor_tensor(
                celli[:], cellf[:], boff_t[:, j : j + 1], op=Alu.add
            )

            # ---- winner resolution (last valid write wins) ----
            # r_eff = valid ? r : -1 so invalid boxes never match any cell
            reff = work.tile([P2, 1], FP32, tag="reff")
            nc.vector.scalar_tensor_tensor(
                reff[:], rr[:], 1.0, valid[:], op0=Alu.add, op1=Alu.mult
            )
            nc.vector.tensor_scalar(reff[:], reff[:], -1.0, None, op0=Alu.add)

            qT = psum.tile([P2, 2 * P2], FP32, tag="qT", space="PSUM")
            nc.tensor.transpose(
                qT[:, 0:P2], reff[:].to_broadcast([P2, P2]), ident_t[:]
            )
            nc.tensor.transpose(
                qT[:, P2 : 2 * P2], cc[:].to_broadcast([P2, P2]), ident_t[:]
            )

            sameR = work.tile([P2, P2], FP32, tag="sameR")
            nc.vector.tensor_scalar(
                sameR[:], qT[:, 0:P2], reff[:], None, op0=Alu.is_equal
            )
            sameRC = work.tile([P2, P2], FP32, tag="sameRC")
            nc.vector.scalar_tensor_tensor(
                sameRC[:], qT[:, P2 : 2 * P2], cc[:], sameR[:],
                op0=Alu.is_equal, op1=Alu.mult,
            )
            scrap0 = work.tile([P2, P2], FP32, tag="scrap0")
            coll = work.tile([P2, 1], FP32, tag="coll")
            nc.vector.tensor_tensor(scrap0[:], sameRC[:], ut_t[:], op=Alu.mult)
            nc.vector.tensor_reduce(
                coll[:], scrap0[:], axis=mybir.AxisListType.X, op=Alu.add
            )
            winner = work.tile([P2, 1], FP32, tag="winner")
            nc.vector.scalar_tensor_tensor(
                winner[:], coll[:], 0.0, valid[:], op0=Alu.is_equal, op1=Alu.mult
            )

            # ---- gather the 100 cell blocks: [100, 255] ----
            graw = gpool.tile([P2, ROWLEN], FP32, tag="graw")
            nc.gpsimd.indirect_dma_start(
                out=graw[:],
                out_offset=None,
                in_=x_ap,
                in_offset=bass.IndirectOffsetOnAxis(ap=celli[:, :1], axis=0),
            )

            # ---- per-cell CE pieces ----
            gv = graw[:].rearrange("p (a f) -> p a f", a=3)[:, :, 5:]
            ex = gpool.tile([P2, 3 * NC_CLS], FP32, tag="ex")
            nc.scalar.activation(
                ex[:].rearrange("p (a f) -> p a f", f=NC_CLS), gv, Act.Exp
            )
            se = work.tile([P2, 3], FP32, tag="se")
            nc.vector.tensor_reduce(
                se[:], ex[:].rearrange("p (a f) -> p a f", f=NC_CLS),
                axis=mybir.AxisListType.X, op=Alu.add,
            )
            lse = work.tile([P2, 3], FP32, tag="lse")
            nc.scalar.activation(lse[:], se[:], Act.Ln)
            s3 = work.tile([P2, 1], FP32, tag="s3")
            nc.vector.tensor_reduce(
                s3[:], lse[:], axis=mybir.AxisListType.X, op=Alu.add
            )

            # label logit sum over the 3 rows: one-hot dot against cidx
            ohc = work.tile([P2, ROWLEN], FP32, tag="ohc")
            nc.vector.tensor_scalar(ohc[:], cidx_t[:], cls, None, op0=Alu.is_equal)
            scrap1 = work.tile([P2, ROWLEN], FP32, tag="scrap1")
            nc.vector.tensor_tensor(scrap1[:], ohc[:], graw[:], op=Alu.mult)
            g3 = work.tile([P2, 1], FP32, tag="g3")
            nc.vector.tensor_reduce(
                g3[:], scrap1[:], axis=mybir.AxisListType.X, op=Alu.add
            )

            # d = (lse_sum - label_logit_sum); stats cols: num = winner*d, cnt = winner
            d = work.tile([P2, 1], FP32, tag="d")
            nc.vector.tensor_tensor(d[:], s3[:], g3[:], op=Alu.subtract)
            nc.vector.tensor_tensor(
                stats[:, 2 * j : 2 * j + 1], d[:], winner[:], op=Alu.mult
            )
            nc.vector.tensor_copy(stats[:, 2 * j + 1 : 2 * j + 2], winner[:])

        # ---- per-batch partition sums via PE: red[i, 2j+k] = batch 2j+i ----
        red = psumr.tile([2, 4], FP32, tag="red", space="PSUM")
        nc.tensor.matmul(red[:], bsel_t[:], stats[:], start=True, stop=True)
        fin = fpool.tile([2, 4], FP32)
        nc.vector.tensor_copy(fin[:], red[:])
        nc.sync.dma_start(out_ap[:], fin[:])


_CACHE = {}


def _get_compiled():
    if "nc" in _CACHE:
        return _CACHE["nc"]
    nc = bacc.Bacc(
        "TRN2",
        target_bir_lowering=False,
        debug=False,
        enable_asserts=False,
        num_devices=N_CORES,
    )
    x = nc.dram_tensor("xflat", [B_CORE * CELLS, ROWLEN], FP32, kind="ExternalInput")
    t = nc.dram_tensor("tgt", [B_CORE, M, 5], FP32, kind="ExternalInput")
    cidx = nc.dram_tensor("cidx", [P2, ROWLEN], FP32, kind="ExternalInput")
    ut2 = nc.dram_tensor("ut2", [P2, P2], FP32, kind="ExternalInput")
    ident = nc.dram_tensor("ident", [P2, P2], FP32, kind="ExternalInput")
    boff = nc.dram_tensor("boff", [P2, 2], FP32, kind="ExternalInput")
    bsel = nc.dram_tensor("bsel", [P2, 2], FP32, kind="ExternalInput")
    out = nc.dram_tensor("statsout", [2, 4], FP32, kind="ExternalOutput")

    with tile.TileContext(nc) as tc:
        _build_kernel_body(
            tc, x.ap(), t.ap(), out.ap(), cidx.ap(), ut2.ap(), ident.ap(),
            boff.ap(), bsel.ap(),
        )
    nc.compile()
    _CACHE["nc"] = nc
    return nc


def _finish(stats_list):
    """Host: per-batch mean, then mean over global batch (float64)."""
    total = 0.0
    for st in stats_list:
        st = np.asarray(st, dtype=np.float64)  # [2, 4]
        for j in range(2):
            for i in range(2):
                num = st[i, 2 * j]
                cnt = st[i, 2 * j + 1]
                total += num / max(3.0 * cnt, 1.0)
    return total / B


def _run(output, targets, trace=False):
    nc = _get_compiled()
    consts = _host_consts()
    output = np.ascontiguousarray(output, dtype=np.float32)
    targets = np.ascontiguousarray(targets, dtype=np.float32)
    in_maps = []
    for k in range(N_CORES):
        in_maps.append(
            {
                "xflat": output[k * B_CORE : (k + 1) * B_CORE].reshape(
                    B_CORE * CELLS, ROWLEN
                ),
                "tgt": targets[k * B_CORE : (k + 1) * B_CORE],
                **consts,
            }
        )
    res = run_bass_kernel_spmd(nc, in_maps, core_ids=list(range(N_CORES)), trace=trace)
    total = _finish([r["statsout"] for r in res.results])
    return np.float32(total), res


def kernel(output, targets):
    val, _ = _run(output, targets)
    return np.asarray(val, dtype=np.float32)


# revision 11
# speedup vs baseline: 1.2388x; 1.0153x over previous
"""Trainium2 Bass kernel for nn_ClassLoss (YOLO-style classification CE loss).

Strategy: the loss depends only on grid cells hit by valid target boxes
(<=50 cells/batch out of 4096). Each cell corresponds to 3 consecutive
"flat rows" of the [12288, 85] logits block (765 contiguous floats in DRAM).
So instead of streaming 127MB of logits, each core:
  1. loads its 4 batches' targets,
  2. computes per-box (row, col, class, valid), resolves last-write-wins
     duplicates with a pairwise comparison (block-diagonal across batches),
  3. indirect-DMA-gathers the needed cell blocks (two [100, 255] gathers,
     batches stacked in pairs along the partition axis),
  4. computes logsumexp over the 80 classes for the 3 rows of each cell and
     the label logit via a one-hot dot, masked by the winner flags,
  5. reduces to per-batch (loss_sum, cell_count) pairs via a selector matmul.
Host applies the per-batch mean (num / max(3*cnt,1)), sums across cores and
divides by the global batch size (the all-reduce + normalize of the
data-parallel sharding).
"""

import sys

sys.path.insert(0, "/opt/trn_rl_repo")

import numpy as np

import concourse.bass as bass
import concourse.tile as tile
from concourse import bacc, mybir
from concourse.bass_utils import run_bass_kernel_spmd

# Problem constants (hardcoded per harness contract).
B, A, H, W, NC_CLS, M = 32, 3, 64, 64, 80, 50
N_CORES = 8
B_CORE = B // N_CORES          # 4 batches per core
CELLS = H * W                  # 4096 cells per batch
ROWLEN = 3 * (5 + NC_CLS)      # 255 floats per cell (3 anchor rows x 85)
P2 = 2 * M                     # 100 partitions: 2 batches x 50 boxes
FP32 = mybir.dt.float32
I32 = mybir.dt.int32
Alu = mybir.AluOpType
Act = mybir.ActivationFunctionType


def _host_consts():
    # cidx[*, a*85 + k] = k-5 for k in [5,85), else -1 (never matches a class)
    cidx = np.full((P2, ROWLEN), -1.0, dtype=np.float32)
    for a in range(3):
        cidx[:, a * 85 + 5 : (a + 1) * 85] = np.arange(NC_CLS, dtype=np.float32)
    # ut2[p, q] = 1 iff same 50-block and q%50 > p%50 (strictly-later box)
    blk = np.arange(P2) // M
    mi = np.arange(P2) % M
    ut2 = ((blk[:, None] == blk[None, :]) & (mi[None, :] > mi[:, None])).astype(
        np.float32
    )
    ident = np.eye(P2, dtype=np.float32)
    # cell offset per partition, per pair: batch = 2*j + p//50
    boff = np.empty((P2, 2), dtype=np.float32)
    for j in range(2):
        boff[:M, j] = (2 * j) * CELLS
        boff[M:, j] = (2 * j + 1) * CELLS
    # block selector for per-batch partition sums
    bsel = np.zeros((P2, 2), dtype=np.float32)
    bsel[:M, 0] = 1.0
    bsel[M:, 1] = 1.0
    return {"cidx": cidx, "ut2": ut2, "ident": ident, "boff": boff, "bsel": bsel}


def _build_kernel_body(tc, x_ap, t_ap, out_ap, cidx_ap, ut_ap, ident_ap, boff_ap, bsel_ap):
    nc = tc.nc
    from contextlib import ExitStack

    ctx = ExitStack()
    with ctx:
        consts = ctx.enter_context(tc.tile_pool(name="consts", bufs=1))
        work = ctx.enter_context(tc.tile_pool(name="work", bufs=3))
        gpool = ctx.enter_context(tc.tile_pool(name="gather", bufs=2))
        psum = ctx.enter_context(tc.tile_pool(name="psum", bufs=2, space="PSUM"))
        psumr = ctx.enter_context(tc.tile_pool(name="psumr", bufs=1, space="PSUM"))
        fpool = ctx.enter_context(tc.tile_pool(name="final", bufs=1))

        # ---- constants / persistent tiles ----
        cidx_t = consts.tile([P2, ROWLEN], FP32)
        nc.sync.dma_start(cidx_t[:], cidx_ap[:])
        ut_t = consts.tile([P2, P2], FP32)
        nc.sync.dma_start(ut_t[:], ut_ap[:])
        ident_t = consts.tile([P2, P2], FP32)
        nc.sync.dma_start(ident_t[:], ident_ap[:])
        boff_t = consts.tile([P2, 2], FP32)
        nc.sync.dma_start(boff_t[:], boff_ap[:])
        bsel_t = consts.tile([P2, 2], FP32)
        nc.sync.dma_start(bsel_t[:], bsel_ap[:])

        stats = fpool.tile([P2, 4], FP32)  # (num, cnt) per pair-column

        # all targets: [100, 2, 5]; partition p = batch-in-pair p//50, box p%50
        tgt_t = consts.tile([P2, 2 * 5], FP32)
        nc.sync.dma_start(
            tgt_t[:].rearrange("p (j f) -> p j f", f=5),
            t_ap.rearrange("(j bb) m f -> (bb m) j f", j=2),
        )

        MAGIC = 8388608.0  # 2^23

        for j in range(2):
            Tb = tgt_t[:].rearrange("p (j f) -> p j f", f=5)[:, j, :]
            cls = Tb[:, 0:1]

            # valid[m] = sum(|t|) > 0
            val1 = work.tile([P2, 1], FP32, tag="val1")
            nc.vector.tensor_reduce(
                val1[:], Tb, axis=mybir.AxisListType.X, op=Alu.add,
                apply_absolute_value=True,
            )
            valid = work.tile([P2, 1], FP32, tag="valid")
            nc.vector.tensor_scalar(valid[:], val1[:], 0.0, None, op0=Alu.is_gt)

            # (c, r) = floor((x, y)*64) fused on [100, 2]: exact branchless
            # floor via ri = RNE(v) (magic add/sub), floor = ri - (ri > v)
            v2 = work.tile([P2, 2], FP32, tag="v2")
            nc.vector.tensor_scalar(v2[:], Tb[:, 1:3], 64.0, None, op0=Alu.mult)
            ri2 = work.tile([P2, 2], FP32, tag="ri2")
            nc.vector.tensor_scalar(ri2[:], v2[:], MAGIC, None, op0=Alu.add)
            nc.vector.tensor_scalar(ri2[:], ri2[:], MAGIC, None, op0=Alu.subtract)
            corr2 = work.tile([P2, 2], FP32, tag="corr2")
            nc.vector.tensor_tensor(corr2[:], ri2[:], v2[:], op=Alu.is_gt)
            fl2 = work.tile([P2, 2], FP32, tag="fl2")
            nc.vector.tensor_tensor(fl2[:], ri2[:], corr2[:], op=Alu.subtract)
            cc, rr = fl2[:, 0:1], fl2[:, 1:2]

            # cell = r*64 + c + batch_offset
            cellf = work.tile([P2, 1], FP32, tag="cellf")
            nc.vector.scalar_tensor_tensor(
                cellf[:], rr, 64.0, cc, op0=Alu.mult, op1=Alu.add
            )
            celli = work.tile([P2, 1], I32, tag="celli")
            nc.vector.tensor_tensor(
                celli[:], cellf[:], boff_t[:, j : j + 1], op=Alu.add
            )

            # ---- gather the 100 cell blocks [100, 255] ASAP so the DMA and
            # exp overlap the winner resolution below ----
            graw = gpool.tile([P2, ROWLEN], FP32, tag="graw")
            nc.gpsimd.indirect_dma_start(
                out=graw[:],
                out_offset=None,
                in_=x_ap,
                in_offset=bass.IndirectOffsetOnAxis(ap=celli[:, :1], axis=0),
            )
            gv = graw[:].rearrange("p (a f) -> p a f", a=3)[:, :, 5:]
            ex = gpool.tile([P2, 3 * NC_CLS], FP32, tag="ex")
            nc.scalar.activation(
                ex[:].rearrange("p (a f) -> p a f", f=NC_CLS), gv, Act.Exp
            )

            # ---- winner resolution (last valid write wins) ----
            # key = valid ? cell : -1 so invalid boxes never match any cell
            key = work.tile([P2, 1], FP32, tag="key")
            nc.vector.scalar_tensor_tensor(
                key[:], cellf[:], 1.0, valid[:], op0=Alu.add, op1=Alu.mult
            )
            nc.vector.tensor_scalar(key[:], key[:], -1.0, None, op0=Alu.add)

            qT = psum.tile([P2, P2], FP32, tag="qT", space="PSUM")
            nc.tensor.transpose(qT[:], key[:].to_broadcast([P2, P2]), ident_t[:])

            same = work.tile([P2, P2], FP32, tag="same")
            nc.vector.tensor_scalar(same[:], qT[:], key[:], None, op0=Alu.is_equal)
            scrap0 = work.tile([P2, P2], FP32, tag="scrap0")
            coll = work.tile([P2, 1], FP32, tag="coll")
            nc.vector.tensor_tensor(scrap0[:], same[:], ut_t[:], op=Alu.mult)
            nc.vector.tensor_reduce(
                coll[:], scrap0[:], axis=mybir.AxisListType.X, op=Alu.add
            )
            winner = work.tile([P2, 1], FP32, tag="winner")
            nc.vector.scalar_tensor_tensor(
                winner[:], coll[:], 0.0, valid[:], op0=Alu.is_equal, op1=Alu.mult
            )

            # ---- per-cell CE pieces ----
            se = work.tile([P2, 3], FP32, tag="se")
            nc.vector.tensor_reduce(
                se[:], ex[:].rearrange("p (a f) -> p a f", f=NC_CLS),
                axis=mybir.AxisListType.X, op=Alu.add,
            )
            lse = work.tile([P2, 3], FP32, tag="lse")
            nc.scalar.activation(lse[:], se[:], Act.Ln)
            s3 = work.tile([P2, 1], FP32, tag="s3")
            nc.vector.tensor_reduce(
                s3[:], lse[:], axis=mybir.AxisListType.X, op=Alu.add
            )

            # label logit sum over the 3 rows: one-hot dot against cidx
            ohc = work.tile([P2, ROWLEN], FP32, tag="ohc")
            nc.vector.tensor_scalar(ohc[:], cidx_t[:], cls, None, op0=Alu.is_equal)
            scrap1 = work.tile([P2, ROWLEN], FP32, tag="scrap1")
            nc.vector.tensor_tensor(scrap1[:], ohc[:], graw[:], op=Alu.mult)
            g3 = work.tile([P2, 1], FP32, tag="g3")
            nc.vector.tensor_reduce(
                g3[:], scrap1[:], axis=mybir.AxisListType.X, op=Alu.add
            )

            # d = (lse_sum - label_logit_sum); stats cols: num = winner*d, cnt = winner
            d = work.tile([P2, 1], FP32, tag="d")
            nc.vector.tensor_tensor(d[:], s3[:], g3[:], op=Alu.subtract)
            nc.vector.tensor_tensor(
                stats[:, 2 * j : 2 * j + 1], d[:], winner[:], op=Alu.mult
            )
            nc.vector.tensor_copy(stats[:, 2 * j + 1 : 2 * j + 2], winner[:])

        # ---- per-batch partition sums via PE: red[i, 2j+k] = batch 2j+i ----
        red = psumr.tile([2, 4], FP32, tag="red", space="PSUM")
        nc.tensor.matmul(red[:], bsel_t[:], stats[:], start=True, stop=True)
        fin = fpool.tile([2, 4], FP32)
        nc.vector.tensor_copy(fin[:], red[:])
        nc.sync.dma_start(out_ap[:], fin[:])


_CACHE = {}


def _get_compiled():
    if "nc" in _CACHE:
        return _CACHE["nc"]
    nc = bacc.Bacc(
        "TRN2",
        target_bir_lowering=False,
        debug=False,
        enable_asserts=False,
        num_devices=N_CORES,
    )
    x = nc.dram_tensor("xflat", [B_CORE * CELLS, ROWLEN], FP32, kind="ExternalInput")
    t = nc.dram_tensor("tgt", [B_CORE, M, 5], FP32, kind="ExternalInput")
    cidx = nc.dram_tensor("cidx", [P2, ROWLEN], FP32, kind="ExternalInput")
    ut2 = nc.dram_tensor("ut2", [P2, P2], FP32, kind="ExternalInput")
    ident = nc.dram_tensor("ident", [P2, P2], FP32, kind="ExternalInput")
    boff = nc.dram_tensor("boff", [P2, 2], FP32, kind="ExternalInput")
    bsel = nc.dram_tensor("bsel", [P2, 2], FP32, kind="ExternalInput")
    out = nc.dram_tensor("statsout", [2, 4], FP32, kind="ExternalOutput")

    with tile.TileContext(nc) as tc:
        _build_kernel_body(
            tc, x.ap(), t.ap(), out.ap(), cidx.ap(), ut2.ap(), ident.ap(),
            boff.ap(), bsel.ap(),
        )
    nc.compile()
    _CACHE["nc"] = nc
    return nc


def _finish(stats_list):
    """Host: per-batch mean, then mean over global batch (float64)."""
    total = 0.0
    for st in stats_list:
        st = np.asarray(st, dtype=np.float64)  # [2, 4]
        for j in range(2):
            for i in range(2):
                num = st[i, 2 * j]
                cnt = st[i, 2 * j + 1]
                total += num / max(3.0 * cnt, 1.0)
    return total / B


def _run(output, targets, trace=False):
    nc = _get_compiled()
    consts = _host_consts()
    output = np.ascontiguousarray(output, dtype=np.float32)
    targets = np.ascontiguousarray(targets, dtype=np.float32)
    in_maps = []
    for k in range(N_CORES):
        in_maps.append(
            {
                "xflat": output[k * B_CORE : (k + 1) * B_CORE].reshape(
                    B_CORE * CELLS, ROWLEN
                ),
                "tgt": targets[k * B_CORE : (k + 1) * B_CORE],
                **consts,
            }
        )
    res = run_bass_kernel_spmd(nc, in_maps, core_ids=list(range(N_CORES)), trace=trace)
    total = _finish([r["statsout"] for r in res.results])
    return np.float32(total), res


def kernel(output, targets):
    val, _ = _run(output, targets)
    return np.asarray(val, dtype=np.float32)
